# revision 14
# baseline (speedup 1.0000x reference)
"""Akima spline interpolation kernel for Trainium2 (8 NeuronCores, data parallel).

Strategy:
  - The reference output is f(x) = akima_spline(x) for x in [0,1). Host
    fits an L2-optimal straight line to f on each of 511 uniform bins of
    width 1/510 centered at j/510 (bin j covers [(j-0.5)/510,
    (j+0.5)/510)): f(x) ~ A[j] + B[j]*w with w = 510*x - j in [-.5,.5).
    A and B are rounded to bf16 and packed into one uint32 per bin
    (A low half, B high half) - a 512-entry table, which fits the GPSIMD
    gather's 512-element pool-buffer window so each tile needs exactly
    one single-window gather. Measured end-to-end rel L2 error of this
    model (including bf16 rounding): ~5.1e-3 (budget 2e-2).
  - Device per tile: rbig = act(copy, x, scale=510, bias=M) rounds 510*x
    to the nearest integer via the magic constant M = 1.5*2^23; two more
    act copies with bias=-M produce the index as uint32 and as float;
    one gather fetches the packed word; the vector engine computes
    w = 510*x - idxf (scalar_tensor_tensor) and evaluates A + B*w as two
    tensor_tensor ops on stride-2 bf16 views of the gathered word.
    Per tile: scalar 3 ops, vector 3 ops, GPSIMD 1 gather - roughly
    balanced, with the gather the slight bottleneck.
  - Sharding: pure data parallel on the leading dim (4 of 32 planes per
    core); the 2KB table is replicated to all partitions of every core.
"""
import base64
import json
import sys

import numpy as np

if "/opt/trn_rl_repo" not in sys.path:
    sys.path.insert(0, "/opt/trn_rl_repo")

NODES = 256
N_CORES = 8
ROWS = 128
COLS = 4 * 1024 * 1024 // ROWS  # per-core shard [128, 32768]
F_TILE = 2048
NSEG = 510                      # bins per unit; table has NSEG+1 entries
TAB_N = 512                     # padded table length (= pool buffer window)
MAGIC = float(np.float32(1.5 * 2.0 ** 23))

# ----------------------------------------------------------------------------
# Host-side table construction
# ----------------------------------------------------------------------------


def _akima_slopes_f64(value):
    h = 1.0 / (NODES - 1)
    v = value.astype(np.float64)
    m = (v[1:] - v[:-1]) / h
    m_m1 = 2.0 * m[0] - m[1]
    m_m2 = 2.0 * m_m1 - m[0]
    m_p1 = 2.0 * m[-1] - m[-2]
    m_p2 = 2.0 * m_p1 - m[-1]
    me = np.concatenate([[m_m2, m_m1], m, [m_p1, m_p2]])
    w1 = np.abs(me[3:] - me[2:-1])
    w2 = np.abs(me[1:-2] - me[:-3])
    mi_1 = me[1:-2]
    mi = me[2:-1]
    denom = w1 + w2
    safe = np.where(denom > 0, denom, 1.0)
    return np.where(denom > 0, (w1 * mi_1 + w2 * mi) / safe, 0.5 * (mi_1 + mi))


def _spline_f64(x, value, s):
    h = 1.0 / (NODES - 1)
    v = value.astype(np.float64)
    x = np.clip(x, 0.0, 1.0)
    t = x / h
    idx = np.clip(np.floor(t).astype(np.int64), 0, NODES - 2)
    u = t - idx
    v0 = v[idx]
    v1 = v[idx + 1]
    s0 = s[idx]
    s1 = s[idx + 1]
    u2 = u * u
    u3 = u2 * u
    return ((2 * u3 - 3 * u2 + 1) * v0 + (u3 - 2 * u2 + u) * h * s0
            + (-2 * u3 + 3 * u2) * v1 + (u3 - u2) * h * s1)


def _build_table(value):
    import ml_dtypes
    s = _akima_slopes_f64(value)
    SS = 64
    j = np.arange(NSEG + 1)
    offs = (np.arange(SS) + 0.5) / SS - 0.5
    xs = (j[:, None] + offs[None, :]) / NSEG
    xs = np.clip(xs, 0.0, 1.0 - 1e-12)
    f = _spline_f64(xs.ravel(), value, s).reshape(NSEG + 1, SS)
    A = f.mean(axis=1)
    B = 12.0 * (f * offs[None, :]).mean(axis=1)
    Ab = A.astype(ml_dtypes.bfloat16).view(np.uint16).astype(np.uint32)
    Bb = B.astype(ml_dtypes.bfloat16).view(np.uint16).astype(np.uint32)
    tab = np.zeros(TAB_N, dtype=np.uint32)
    tab[:NSEG + 1] = Ab | (Bb << 16)
    return tab


# ----------------------------------------------------------------------------
# NKI kernel
# ----------------------------------------------------------------------------


def _make_nki_kernel():
    import neuronxcc.nki.language as nl
    import neuronxcc.nki.isa as nisa
    from neuronxcc.nki.isa.constants import dge_mode

    n_tiles = COLS // F_TILE

    def akima_kernel(inputs):
        x, table = inputs[0], inputs[1]
        out = nl.ndarray(shape=[ROWS, COLS], dtype=nl.float32, buffer=nl.shared_hbm)
        tab_sb = nl.load(table)
        i_p = nl.arange(ROWS)[:, None]
        i_f = nl.arange(F_TILE)[None, :]
        magic_bias = nisa.memset((ROWS, 1), MAGIC, nl.float32)
        neg_magic_bias = nisa.memset((ROWS, 1), -MAGIC, nl.float32)

        # Explicit rotating SBUF buffers: without them the allocator's
        # address reuse creates WAR hazards that serialize consecutive
        # tiles.
        NBUF = 2

        def mkbufs():
            return dict(
                x=nl.ndarray(shape=[ROWS, F_TILE], dtype=nl.float32, buffer=nl.sbuf),
                rbig=nl.ndarray(shape=[ROWS, F_TILE], dtype=nl.float32, buffer=nl.sbuf),
                idx=nl.ndarray(shape=[ROWS, F_TILE], dtype=nl.uint32, buffer=nl.sbuf),
                idxf=nl.ndarray(shape=[ROWS, F_TILE], dtype=nl.float32, buffer=nl.sbuf),
                g=nl.ndarray(shape=[ROWS, F_TILE], dtype=nl.uint32, buffer=nl.sbuf),
                w=nl.ndarray(shape=[ROWS, F_TILE], dtype=nl.float32, buffer=nl.sbuf),
                m=nl.ndarray(shape=[ROWS, F_TILE], dtype=nl.float32, buffer=nl.sbuf),
                r=nl.ndarray(shape=[ROWS, F_TILE], dtype=nl.float32, buffer=nl.sbuf),
            )

        bufs = [mkbufs() for _ in range(NBUF)]

        for t in range(n_tiles):
            B = bufs[t % NBUF]
            sl = slice(t * F_TILE, (t + 1) * F_TILE)
            # HWDGE DMAs: keep descriptor generation off GPSIMD (SWDGE's
            # Q7 descgen and descriptor rings contend with the gather and
            # with 2-port vector ops for SBUF ports).
            nisa.dma_copy(src=x[:, sl], dst=B['x'][i_p, i_f],
                          dge_mode=dge_mode.hwdge)
            x_sb = B['x'][i_p, i_f]
            B['rbig'][i_p, i_f] = nisa.activation(
                np.copy, x_sb, bias=magic_bias, scale=float(NSEG))
            B['idx'][i_p, i_f] = nisa.activation(
                np.copy, B['rbig'][i_p, i_f], bias=neg_magic_bias, dtype=nl.uint32)
            B['idxf'][i_p, i_f] = nisa.activation(
                np.copy, B['rbig'][i_p, i_f], bias=neg_magic_bias)
            B['g'][i_p, i_f] = nl.gather_flattened(
                data=tab_sb, indices=B['idx'][i_p, i_f])
            B['w'][i_p, i_f] = nisa.scalar_tensor_tensor(
                data=x_sb, op0=np.multiply, operand0=float(NSEG),
                op1=np.subtract, operand1=B['idxf'][i_p, i_f])
            gb = B['g'].view(nl.bfloat16)  # [P, 2F]: A at even, B at odd
            B['m'][i_p, i_f] = nisa.tensor_tensor(
                gb[i_p, i_f * 2 + 1], B['w'][i_p, i_f], np.multiply,
                dtype=nl.float32)
            B['r'][i_p, i_f] = nisa.tensor_tensor(
                B['m'][i_p, i_f], gb[i_p, i_f * 2], np.add,
                dtype=nl.float32)
            nisa.dma_copy(src=B['r'][i_p, i_f], dst=out[:, sl],
                          dge_mode=dge_mode.hwdge)
        return [out]

    return akima_kernel


# ----------------------------------------------------------------------------
# jax integration (AwsNeuronCustomNativeKernel custom call, SPMD over 8 cores)
# ----------------------------------------------------------------------------

_EXEC_CACHE = {}


def _build_executor():
    if "exec" in _EXEC_CACHE:
        return _EXEC_CACHE["exec"]

    import functools
    import jax
    from jax.interpreters import mlir
    from jax._src.interpreters.mlir import custom_call as _mlir_custom_call
    from jax.sharding import Mesh, PartitionSpec
    from jax.experimental.shard_map import shard_map
    from concourse.bass2jax import install_neuronx_cc_hook

    install_neuronx_cc_hook()

    def raw_nki(func):
        # concourse.nki.raw_nki with platform_target='trn2' (the default
        # CompileOpts says trn1, which rejects HWDGE dma_copy).
        from neuronxcc.nki.compiler.backends.neuron.CompileOpts import CompileOpts
        from neuronxcc.nki.compiler.backends.neuron.KernelBuilder import NeuronCodegen
        from neuronxcc.nki.compiler.backends.neuron.nki_ctx import nki_ctx
        from neuronxcc.nki.compiler.backends.neuron.tensors import TensorRef
        from neuronxcc.starfish.penguin.ir.Function import Function
        from neuronxcc.starfish.penguin.ir.OptLevel import OptLevel

        @functools.wraps(func)
        def wrapper(inputs):
            code = Function(name="func", opt_level=OptLevel.default_level)
            bb = code.addBasicBlock()
            opts = CompileOpts(platform_target="trn2")
            with NeuronCodegen.new_ctx(cu=code, curstmt=bb, opts=opts) as ctx:
                with ctx.kernel_scope(
                    ctx.function, py_func=func, spmd_block=ctx.builder.curstmt
                ) as scope:
                    nki_inputs = []
                    for i, inp in enumerate(inputs):
                        tensor = nki_ctx().add_parameter(
                            name=f"input{i}",
                            shape=list(inp.shape),
                            dtype=inp.dtype,
                            is_mutable=False,
                        )
                        tensor.isInput = True
                        nki_inputs.append(TensorRef(tensor))
                    outputs = func(nki_inputs)
                    scope.add_kernel_return_values(list(outputs))
                ctx.finalize_kernel(scope)
            return code

        return wrapper

    nki_func = _make_nki_kernel()

    prim = jax.extend.core.Primitive("akima_exec")
    prim.multiple_results = True

    @prim.def_abstract_eval
    def _abs(*_, **__):
        return (jax.core.ShapedArray((ROWS, COLS), np.float32),)

    def _layouts(shapes):
        return [list(reversed(range(len(s)))) for s in shapes]

    def _lowering(ctx, *in_nodes):
        from neuronxcc.starfish.penguin.ir.NativeKernel import KERNEL_VERSION

        result_types = [mlir.aval_to_ir_type(a) for a in ctx.avals_out]
        code = raw_nki(nki_func)(list(ctx.avals_in))
        config = {
            "kernel_version": KERNEL_VERSION,
            "func_literal": code.serialize_ir_string("akima_kernel_ir"),
            "grid": [],
            "func_name": "akima_kernel",
            "has_collectives": False,
            "mac_count": 0,
            "tiled": False,
        }
        dumped = base64.b64encode(json.dumps(config).encode()).decode()
        return _mlir_custom_call(
            "AwsNeuronCustomNativeKernel",
            operands=list(in_nodes),
            result_types=result_types,
            operand_layouts=_layouts(a.shape for a in ctx.avals_in),
            result_layouts=_layouts(a.shape for a in ctx.avals_out),
            backend_config=dumped,
        ).results

    mlir.register_lowering(prim, _lowering, platform="neuron")

    devices = jax.devices()[:N_CORES]
    mesh = Mesh(np.asarray(devices), ("core",))

    def _body(x_shard, tab_shard):
        return prim.bind(x_shard, tab_shard)[0]

    sharded = jax.jit(shard_map(
        _body, mesh=mesh,
        in_specs=(PartitionSpec("core"), PartitionSpec("core")),
        out_specs=PartitionSpec("core"),
        check_rep=False,
    ))

    _EXEC_CACHE["exec"] = sharded
    return sharded


# ----------------------------------------------------------------------------
# Public entry point
# ----------------------------------------------------------------------------


def kernel(input: np.ndarray, value: np.ndarray) -> np.ndarray:
    input = np.ascontiguousarray(np.asarray(input, dtype=np.float32))
    value = np.asarray(value, dtype=np.float32)
    assert input.shape == (32, 1024, 1024), input.shape

    tab = _build_table(value)
    table = np.broadcast_to(tab, (ROWS, TAB_N)).copy()

    sharded = _build_executor()

    # shard on the leading dim: core i gets planes [4i, 4i+4)
    x_global = input.reshape(N_CORES * ROWS, COLS)
    tab_global = np.tile(table, (N_CORES, 1))

    out = sharded(x_global, tab_global)
    return np.asarray(out).reshape(32, 1024, 1024)


if __name__ == "__main__":
    inp = np.load("cache/input.npy")
    val = np.load("cache/value.npy")
    out = kernel(input=inp, value=val)
    exp = np.load("cache/expected.npy")
    err = out.astype(np.float64) - exp.astype(np.float64)
    print("rel_l2:", np.linalg.norm(err) / np.linalg.norm(exp))


# revision 18
# speedup vs baseline: 1.1463x; 1.1463x over previous
"""Akima spline interpolation kernel for Trainium2 (8 NeuronCores, data parallel).

Strategy:
  - The reference output is f(x) = akima_spline(x) for x in [0,1). Host
    fits an L2-optimal straight line to f on each of 511 uniform bins of
    width 1/510 centered at j/510 (bin j covers [(j-0.5)/510,
    (j+0.5)/510)): f(x) ~ A[j] + B[j]*w with w = 510*x - j in [-.5,.5).
    A and B are rounded to bf16 and packed into one uint32 per bin
    (A low half, B high half) - a 512-entry table, which fits the GPSIMD
    gather's 512-element pool-buffer window so each tile needs exactly
    one single-window gather. Measured end-to-end rel L2 error of this
    model (including bf16 rounding): ~5.1e-3 (budget 2e-2).
  - Device per tile: rbig = act(copy, x, scale=510, bias=M) rounds 510*x
    to the nearest integer via the magic constant M = 1.5*2^23; two more
    act copies with bias=-M produce the index as uint32 and as float;
    one gather fetches the packed word; the vector engine computes
    w = 510*x - idxf (scalar_tensor_tensor) and evaluates A + B*w as two
    tensor_tensor ops on stride-2 bf16 views of the gathered word.
    Per tile: scalar 3 ops, vector 3 ops, GPSIMD 1 gather - roughly
    balanced, with the gather the slight bottleneck.
  - Sharding: pure data parallel on the leading dim (4 of 32 planes per
    core); the 2KB table is replicated to all partitions of every core.
"""
import base64
import json
import sys

import numpy as np

if "/opt/trn_rl_repo" not in sys.path:
    sys.path.insert(0, "/opt/trn_rl_repo")

NODES = 256
N_CORES = 8
ROWS = 128
COLS = 4 * 1024 * 1024 // ROWS  # per-core shard [128, 32768]
F_TILE = 2048
NSEG = 510                      # bins per unit; table has NSEG+1 entries
TAB_N = 512                     # padded table length (= pool buffer window)
MAGIC = float(np.float32(1.5 * 2.0 ** 23))

# ----------------------------------------------------------------------------
# Host-side table construction
# ----------------------------------------------------------------------------


def _akima_slopes_f64(value):
    h = 1.0 / (NODES - 1)
    v = value.astype(np.float64)
    m = (v[1:] - v[:-1]) / h
    m_m1 = 2.0 * m[0] - m[1]
    m_m2 = 2.0 * m_m1 - m[0]
    m_p1 = 2.0 * m[-1] - m[-2]
    m_p2 = 2.0 * m_p1 - m[-1]
    me = np.concatenate([[m_m2, m_m1], m, [m_p1, m_p2]])
    w1 = np.abs(me[3:] - me[2:-1])
    w2 = np.abs(me[1:-2] - me[:-3])
    mi_1 = me[1:-2]
    mi = me[2:-1]
    denom = w1 + w2
    safe = np.where(denom > 0, denom, 1.0)
    return np.where(denom > 0, (w1 * mi_1 + w2 * mi) / safe, 0.5 * (mi_1 + mi))


def _spline_f64(x, value, s):
    h = 1.0 / (NODES - 1)
    v = value.astype(np.float64)
    x = np.clip(x, 0.0, 1.0)
    t = x / h
    idx = np.clip(np.floor(t).astype(np.int64), 0, NODES - 2)
    u = t - idx
    v0 = v[idx]
    v1 = v[idx + 1]
    s0 = s[idx]
    s1 = s[idx + 1]
    u2 = u * u
    u3 = u2 * u
    return ((2 * u3 - 3 * u2 + 1) * v0 + (u3 - 2 * u2 + u) * h * s0
            + (-2 * u3 + 3 * u2) * v1 + (u3 - u2) * h * s1)


def _build_table(value):
    import ml_dtypes
    s = _akima_slopes_f64(value)
    SS = 64
    j = np.arange(NSEG + 1)
    offs = (np.arange(SS) + 0.5) / SS - 0.5
    xs = (j[:, None] + offs[None, :]) / NSEG
    xs = np.clip(xs, 0.0, 1.0 - 1e-12)
    f = _spline_f64(xs.ravel(), value, s).reshape(NSEG + 1, SS)
    A = f.mean(axis=1)
    B = 12.0 * (f * offs[None, :]).mean(axis=1)
    Ab = A.astype(ml_dtypes.bfloat16).view(np.uint16).astype(np.uint32)
    Bb = B.astype(ml_dtypes.bfloat16).view(np.uint16).astype(np.uint32)
    tab = np.zeros(TAB_N, dtype=np.uint32)
    tab[:NSEG + 1] = Ab | (Bb << 16)
    return tab


# ----------------------------------------------------------------------------
# NKI kernel
# ----------------------------------------------------------------------------


def _make_nki_kernel():
    import neuronxcc.nki.language as nl
    import neuronxcc.nki.isa as nisa
    from neuronxcc.nki.isa.constants import dge_mode

    n_tiles = COLS // F_TILE

    def akima_kernel(inputs):
        x, table = inputs[0], inputs[1]
        out = nl.ndarray(shape=[ROWS, COLS], dtype=nl.float32, buffer=nl.shared_hbm)
        tab_sb = nl.load(table)
        i_p = nl.arange(ROWS)[:, None]
        i_f = nl.arange(F_TILE)[None, :]
        magic_bias = nisa.memset((ROWS, 1), MAGIC, nl.float32)
        neg_magic_bias = nisa.memset((ROWS, 1), -MAGIC, nl.float32)

        # Explicit rotating SBUF buffers: without them the allocator's
        # address reuse creates WAR hazards that serialize consecutive
        # tiles.
        NBUF = 2

        def mkbufs():
            return dict(
                x=nl.ndarray(shape=[ROWS, F_TILE], dtype=nl.float32, buffer=nl.sbuf),
                rbig=nl.ndarray(shape=[ROWS, F_TILE], dtype=nl.float32, buffer=nl.sbuf),
                idx=nl.ndarray(shape=[ROWS, F_TILE], dtype=nl.uint32, buffer=nl.sbuf),
                idxf=nl.ndarray(shape=[ROWS, F_TILE], dtype=nl.float32, buffer=nl.sbuf),
                g=nl.ndarray(shape=[ROWS, F_TILE], dtype=nl.uint32, buffer=nl.sbuf),
                r=nl.ndarray(shape=[ROWS, F_TILE], dtype=nl.float32, buffer=nl.sbuf),
            )

        bufs = [mkbufs() for _ in range(NBUF)]
        # w and m live in PSUM: GPSIMD has no PSUM port, so vector-engine
        # ops with a PSUM operand need only one SBUF port and stop
        # colliding with the gather for the shared POOL/DVE SBUF port.
        # A PSUM tile is one 512-f32 bank, so the vector stage runs in
        # 512-wide chunks. Single-buffered is safe: the vector engine
        # executes its ops in order, so each chunk's writer issues after
        # the previous chunk's readers.
        PCH = 512
        w_ps = nl.ndarray(shape=[ROWS, PCH], dtype=nl.float32, buffer=nl.psum)
        m_ps = nl.ndarray(shape=[ROWS, PCH], dtype=nl.float32, buffer=nl.psum)
        i_c = nl.arange(PCH)[None, :]

        for t in range(n_tiles):
            B = bufs[t % NBUF]
            sl = slice(t * F_TILE, (t + 1) * F_TILE)
            # HWDGE DMAs: keep descriptor generation off GPSIMD (SWDGE's
            # Q7 descgen and descriptor rings contend with the gather and
            # with 2-port vector ops for SBUF ports).
            nisa.dma_copy(src=x[:, sl], dst=B['x'][i_p, i_f],
                          dge_mode=dge_mode.hwdge)
            x_sb = B['x'][i_p, i_f]
            B['rbig'][i_p, i_f] = nisa.activation(
                np.copy, x_sb, bias=magic_bias, scale=float(NSEG))
            B['idx'][i_p, i_f] = nisa.activation(
                np.copy, B['rbig'][i_p, i_f], bias=neg_magic_bias, dtype=nl.uint32)
            B['idxf'][i_p, i_f] = nisa.activation(
                np.copy, B['rbig'][i_p, i_f], bias=neg_magic_bias)
            B['g'][i_p, i_f] = nl.gather_flattened(
                data=tab_sb, indices=B['idx'][i_p, i_f])
            gb = B['g'].view(nl.bfloat16)  # [P, 2F]: A at even, B at odd
            for c in range(F_TILE // PCH):
                cf = c * PCH + i_c
                w_ps[i_p, i_c] = nisa.scalar_tensor_tensor(
                    data=B['x'][i_p, cf], op0=np.multiply,
                    operand0=float(NSEG), op1=np.subtract,
                    operand1=B['idxf'][i_p, cf])
                m_ps[i_p, i_c] = nisa.tensor_tensor(
                    gb[i_p, cf * 2 + 1], w_ps[i_p, i_c], np.multiply,
                    dtype=nl.float32)
                B['r'][i_p, cf] = nisa.tensor_tensor(
                    m_ps[i_p, i_c], gb[i_p, cf * 2], np.add,
                    dtype=nl.float32)
            nisa.dma_copy(src=B['r'][i_p, i_f], dst=out[:, sl],
                          dge_mode=dge_mode.hwdge)
        return [out]

    return akima_kernel


# ----------------------------------------------------------------------------
# jax integration (AwsNeuronCustomNativeKernel custom call, SPMD over 8 cores)
# ----------------------------------------------------------------------------

_EXEC_CACHE = {}


def _build_executor():
    if "exec" in _EXEC_CACHE:
        return _EXEC_CACHE["exec"]

    import functools
    import jax
    from jax.interpreters import mlir
    from jax._src.interpreters.mlir import custom_call as _mlir_custom_call
    from jax.sharding import Mesh, PartitionSpec
    from jax.experimental.shard_map import shard_map
    from concourse.bass2jax import install_neuronx_cc_hook

    install_neuronx_cc_hook()

    def raw_nki(func):
        # concourse.nki.raw_nki with platform_target='trn2' (the default
        # CompileOpts says trn1, which rejects HWDGE dma_copy).
        from neuronxcc.nki.compiler.backends.neuron.CompileOpts import CompileOpts
        from neuronxcc.nki.compiler.backends.neuron.KernelBuilder import NeuronCodegen
        from neuronxcc.nki.compiler.backends.neuron.nki_ctx import nki_ctx
        from neuronxcc.nki.compiler.backends.neuron.tensors import TensorRef
        from neuronxcc.starfish.penguin.ir.Function import Function
        from neuronxcc.starfish.penguin.ir.OptLevel import OptLevel

        @functools.wraps(func)
        def wrapper(inputs):
            code = Function(name="func", opt_level=OptLevel.default_level)
            bb = code.addBasicBlock()
            opts = CompileOpts(platform_target="trn2")
            with NeuronCodegen.new_ctx(cu=code, curstmt=bb, opts=opts) as ctx:
                with ctx.kernel_scope(
                    ctx.function, py_func=func, spmd_block=ctx.builder.curstmt
                ) as scope:
                    nki_inputs = []
                    for i, inp in enumerate(inputs):
                        tensor = nki_ctx().add_parameter(
                            name=f"input{i}",
                            shape=list(inp.shape),
                            dtype=inp.dtype,
                            is_mutable=False,
                        )
                        tensor.isInput = True
                        nki_inputs.append(TensorRef(tensor))
                    outputs = func(nki_inputs)
                    scope.add_kernel_return_values(list(outputs))
                ctx.finalize_kernel(scope)
            return code

        return wrapper

    nki_func = _make_nki_kernel()

    prim = jax.extend.core.Primitive("akima_exec")
    prim.multiple_results = True

    @prim.def_abstract_eval
    def _abs(*_, **__):
        return (jax.core.ShapedArray((ROWS, COLS), np.float32),)

    def _layouts(shapes):
        return [list(reversed(range(len(s)))) for s in shapes]

    def _lowering(ctx, *in_nodes):
        from neuronxcc.starfish.penguin.ir.NativeKernel import KERNEL_VERSION

        result_types = [mlir.aval_to_ir_type(a) for a in ctx.avals_out]
        code = raw_nki(nki_func)(list(ctx.avals_in))
        config = {
            "kernel_version": KERNEL_VERSION,
            "func_literal": code.serialize_ir_string("akima_kernel_ir"),
            "grid": [],
            "func_name": "akima_kernel",
            "has_collectives": False,
            "mac_count": 0,
            "tiled": False,
        }
        dumped = base64.b64encode(json.dumps(config).encode()).decode()
        return _mlir_custom_call(
            "AwsNeuronCustomNativeKernel",
            operands=list(in_nodes),
            result_types=result_types,
            operand_layouts=_layouts(a.shape for a in ctx.avals_in),
            result_layouts=_layouts(a.shape for a in ctx.avals_out),
            backend_config=dumped,
        ).results

    mlir.register_lowering(prim, _lowering, platform="neuron")

    devices = jax.devices()[:N_CORES]
    mesh = Mesh(np.asarray(devices), ("core",))

    def _body(x_shard, tab_shard):
        return prim.bind(x_shard, tab_shard)[0]

    sharded = jax.jit(shard_map(
        _body, mesh=mesh,
        in_specs=(PartitionSpec("core"), PartitionSpec("core")),
        out_specs=PartitionSpec("core"),
        check_rep=False,
    ))

    _EXEC_CACHE["exec"] = sharded
    return sharded


# ----------------------------------------------------------------------------
# Public entry point
# ----------------------------------------------------------------------------


def kernel(input: np.ndarray, value: np.ndarray) -> np.ndarray:
    input = np.ascontiguousarray(np.asarray(input, dtype=np.float32))
    value = np.asarray(value, dtype=np.float32)
    assert input.shape == (32, 1024, 1024), input.shape

    tab = _build_table(value)
    table = np.broadcast_to(tab, (ROWS, TAB_N)).copy()

    sharded = _build_executor()

    # shard on the leading dim: core i gets planes [4i, 4i+4)
    x_global = input.reshape(N_CORES * ROWS, COLS)
    tab_global = np.tile(table, (N_CORES, 1))

    out = sharded(x_global, tab_global)
    return np.asarray(out).reshape(32, 1024, 1024)


if __name__ == "__main__":
    inp = np.load("cache/input.npy")
    val = np.load("cache/value.npy")
    out = kernel(input=inp, value=val)
    exp = np.load("cache/expected.npy")
    err = out.astype(np.float64) - exp.astype(np.float64)
    print("rel_l2:", np.linalg.norm(err) / np.linalg.norm(exp))


# revision 22
# speedup vs baseline: 1.1960x; 1.0433x over previous
"""Akima spline interpolation kernel for Trainium2 (8 NeuronCores, data parallel).

Strategy:
  - The reference output is f(x) = akima_spline(x) for x in [0,1). Host
    fits an L2-optimal straight line to f on each of 511 uniform bins of
    width 1/510 centered at j/510 (bin j covers [(j-0.5)/510,
    (j+0.5)/510)): f(x) ~ A[j] + B[j]*w with w = 510*x - j in [-.5,.5).
    A and B are rounded to bf16 and packed into one uint32 per bin
    (A low half, B high half) - a 512-entry table, which fits the GPSIMD
    gather's 512-element pool-buffer window so each tile needs exactly
    one single-window gather. Measured end-to-end rel L2 error of this
    model (including bf16 rounding): ~5.1e-3 (budget 2e-2).
  - Device per tile: rbig = act(copy, x, scale=510, bias=M) rounds 510*x
    to the nearest integer via the magic constant M = 1.5*2^23; two more
    act copies with bias=-M produce the index as uint32 and as float;
    one gather fetches the packed word; the vector engine computes
    w = 510*x - idxf (scalar_tensor_tensor) and evaluates A + B*w as two
    tensor_tensor ops on stride-2 bf16 views of the gathered word.
    Per tile: scalar 3 ops, vector 3 ops, GPSIMD 1 gather - roughly
    balanced, with the gather the slight bottleneck.
  - Sharding: pure data parallel on the leading dim (4 of 32 planes per
    core); the 2KB table is replicated to all partitions of every core.
"""
import base64
import json
import sys

import numpy as np

if "/opt/trn_rl_repo" not in sys.path:
    sys.path.insert(0, "/opt/trn_rl_repo")

NODES = 256
N_CORES = 8
ROWS = 128
COLS = 4 * 1024 * 1024 // ROWS  # per-core shard [128, 32768]
F_TILE = 2048
NSEG = 510                      # bins per unit; table has NSEG+1 entries
TAB_N = 512                     # padded table length (= pool buffer window)
MAGIC = float(np.float32(1.5 * 2.0 ** 23))

# ----------------------------------------------------------------------------
# Host-side table construction
# ----------------------------------------------------------------------------


def _akima_slopes_f64(value):
    h = 1.0 / (NODES - 1)
    v = value.astype(np.float64)
    m = (v[1:] - v[:-1]) / h
    m_m1 = 2.0 * m[0] - m[1]
    m_m2 = 2.0 * m_m1 - m[0]
    m_p1 = 2.0 * m[-1] - m[-2]
    m_p2 = 2.0 * m_p1 - m[-1]
    me = np.concatenate([[m_m2, m_m1], m, [m_p1, m_p2]])
    w1 = np.abs(me[3:] - me[2:-1])
    w2 = np.abs(me[1:-2] - me[:-3])
    mi_1 = me[1:-2]
    mi = me[2:-1]
    denom = w1 + w2
    safe = np.where(denom > 0, denom, 1.0)
    return np.where(denom > 0, (w1 * mi_1 + w2 * mi) / safe, 0.5 * (mi_1 + mi))


def _spline_f64(x, value, s):
    h = 1.0 / (NODES - 1)
    v = value.astype(np.float64)
    x = np.clip(x, 0.0, 1.0)
    t = x / h
    idx = np.clip(np.floor(t).astype(np.int64), 0, NODES - 2)
    u = t - idx
    v0 = v[idx]
    v1 = v[idx + 1]
    s0 = s[idx]
    s1 = s[idx + 1]
    u2 = u * u
    u3 = u2 * u
    return ((2 * u3 - 3 * u2 + 1) * v0 + (u3 - 2 * u2 + u) * h * s0
            + (-2 * u3 + 3 * u2) * v1 + (u3 - u2) * h * s1)


def _build_table(value):
    import ml_dtypes
    s = _akima_slopes_f64(value)
    SS = 64
    j = np.arange(NSEG + 1)
    offs = (np.arange(SS) + 0.5) / SS - 0.5
    xs = (j[:, None] + offs[None, :]) / NSEG
    xs = np.clip(xs, 0.0, 1.0 - 1e-12)
    f = _spline_f64(xs.ravel(), value, s).reshape(NSEG + 1, SS)
    A = f.mean(axis=1)
    B = 12.0 * (f * offs[None, :]).mean(axis=1)
    Ab = A.astype(ml_dtypes.bfloat16).view(np.uint16).astype(np.uint32)
    Bb = B.astype(ml_dtypes.bfloat16).view(np.uint16).astype(np.uint32)
    tab = np.zeros(TAB_N, dtype=np.uint32)
    tab[:NSEG + 1] = Ab | (Bb << 16)
    return tab


# ----------------------------------------------------------------------------
# NKI kernel
# ----------------------------------------------------------------------------


def _make_nki_kernel():
    import neuronxcc.nki.language as nl
    import neuronxcc.nki.isa as nisa
    from neuronxcc.nki.isa.constants import dge_mode

    n_tiles = COLS // F_TILE

    def akima_kernel(inputs):
        x, table = inputs[0], inputs[1]
        out = nl.ndarray(shape=[ROWS, COLS], dtype=nl.float32, buffer=nl.shared_hbm)
        tab_sb = nl.load(table)
        i_p = nl.arange(ROWS)[:, None]
        i_f = nl.arange(F_TILE)[None, :]
        magic_bias = nisa.memset((ROWS, 1), MAGIC, nl.float32)
        neg_magic_bias = nisa.memset((ROWS, 1), -MAGIC, nl.float32)

        # Explicit rotating SBUF buffers: without them the allocator's
        # address reuse creates WAR hazards that serialize consecutive
        # tiles.
        NBUF = 2

        def mkbufs():
            return dict(
                x=nl.ndarray(shape=[ROWS, F_TILE], dtype=nl.float32, buffer=nl.sbuf),
                rbig=nl.ndarray(shape=[ROWS, F_TILE], dtype=nl.float32, buffer=nl.sbuf),
                idx=nl.ndarray(shape=[ROWS, F_TILE], dtype=nl.uint32, buffer=nl.sbuf),
                g=nl.ndarray(shape=[ROWS, F_TILE], dtype=nl.uint32, buffer=nl.sbuf),
                r=nl.ndarray(shape=[ROWS, F_TILE], dtype=nl.float32, buffer=nl.sbuf),
            )

        bufs = [mkbufs() for _ in range(NBUF)]
        # idxf, w and m live in PSUM: GPSIMD has no PSUM port, so
        # vector-engine ops with a PSUM operand need only one SBUF port
        # and stop colliding with the gather for the shared POOL/DVE SBUF
        # port. The scalar engine writes idxf full-width (ACT supports 4K
        # free dim to PSUM); the vector stage runs in 512-f32 one-bank
        # chunks. Single-buffered w/m are safe: the vector engine executes
        # its ops in order, so each chunk's writer issues after the
        # previous chunk's readers.
        PCH = 512
        idxf_ps = [nl.ndarray(shape=[ROWS, PCH], dtype=nl.float32, buffer=nl.psum)
                   for _ in range(2)]
        w_ps = nl.ndarray(shape=[ROWS, PCH], dtype=nl.float32, buffer=nl.psum)
        m_ps = nl.ndarray(shape=[ROWS, PCH], dtype=nl.float32, buffer=nl.psum)
        i_c = nl.arange(PCH)[None, :]

        for t in range(n_tiles):
            B = bufs[t % NBUF]
            sl = slice(t * F_TILE, (t + 1) * F_TILE)
            # HWDGE DMAs: keep descriptor generation off GPSIMD (SWDGE's
            # Q7 descgen and descriptor rings contend with the gather and
            # with 2-port vector ops for SBUF ports).
            nisa.dma_copy(src=x[:, sl], dst=B['x'][i_p, i_f],
                          dge_mode=dge_mode.hwdge)
            x_sb = B['x'][i_p, i_f]
            B['rbig'][i_p, i_f] = nisa.activation(
                np.copy, x_sb, bias=magic_bias, scale=float(NSEG))
            B['idx'][i_p, i_f] = nisa.activation(
                np.copy, B['rbig'][i_p, i_f], bias=neg_magic_bias, dtype=nl.uint32)
            B['g'][i_p, i_f] = nl.gather_flattened(
                data=tab_sb, indices=B['idx'][i_p, i_f])
            gb = B['g'].view(nl.bfloat16)  # [P, 2F]: A at even, B at odd
            for c in range(F_TILE // PCH):
                cf = c * PCH + i_c
                ixp = idxf_ps[c % 2]
                ixp[i_p, i_c] = nisa.activation(
                    np.copy, B['rbig'][i_p, cf], bias=neg_magic_bias)
                w_ps[i_p, i_c] = nisa.scalar_tensor_tensor(
                    data=B['x'][i_p, cf], op0=np.multiply,
                    operand0=float(NSEG), op1=np.subtract,
                    operand1=ixp[i_p, i_c])
                m_ps[i_p, i_c] = nisa.tensor_tensor(
                    gb[i_p, cf * 2 + 1], w_ps[i_p, i_c], np.multiply,
                    dtype=nl.float32)
                B['r'][i_p, cf] = nisa.tensor_tensor(
                    m_ps[i_p, i_c], gb[i_p, cf * 2], np.add,
                    dtype=nl.float32)
            nisa.dma_copy(src=B['r'][i_p, i_f], dst=out[:, sl],
                          dge_mode=dge_mode.hwdge)
        return [out]

    return akima_kernel


# ----------------------------------------------------------------------------
# jax integration (AwsNeuronCustomNativeKernel custom call, SPMD over 8 cores)
# ----------------------------------------------------------------------------

_EXEC_CACHE = {}


def _build_executor():
    if "exec" in _EXEC_CACHE:
        return _EXEC_CACHE["exec"]

    import functools
    import jax
    from jax.interpreters import mlir
    from jax._src.interpreters.mlir import custom_call as _mlir_custom_call
    from jax.sharding import Mesh, PartitionSpec
    from jax.experimental.shard_map import shard_map
    from concourse.bass2jax import install_neuronx_cc_hook

    install_neuronx_cc_hook()

    def raw_nki(func):
        # concourse.nki.raw_nki with platform_target='trn2' (the default
        # CompileOpts says trn1, which rejects HWDGE dma_copy).
        from neuronxcc.nki.compiler.backends.neuron.CompileOpts import CompileOpts
        from neuronxcc.nki.compiler.backends.neuron.KernelBuilder import NeuronCodegen
        from neuronxcc.nki.compiler.backends.neuron.nki_ctx import nki_ctx
        from neuronxcc.nki.compiler.backends.neuron.tensors import TensorRef
        from neuronxcc.starfish.penguin.ir.Function import Function
        from neuronxcc.starfish.penguin.ir.OptLevel import OptLevel

        @functools.wraps(func)
        def wrapper(inputs):
            code = Function(name="func", opt_level=OptLevel.default_level)
            bb = code.addBasicBlock()
            opts = CompileOpts(platform_target="trn2")
            with NeuronCodegen.new_ctx(cu=code, curstmt=bb, opts=opts) as ctx:
                with ctx.kernel_scope(
                    ctx.function, py_func=func, spmd_block=ctx.builder.curstmt
                ) as scope:
                    nki_inputs = []
                    for i, inp in enumerate(inputs):
                        tensor = nki_ctx().add_parameter(
                            name=f"input{i}",
                            shape=list(inp.shape),
                            dtype=inp.dtype,
                            is_mutable=False,
                        )
                        tensor.isInput = True
                        nki_inputs.append(TensorRef(tensor))
                    outputs = func(nki_inputs)
                    scope.add_kernel_return_values(list(outputs))
                ctx.finalize_kernel(scope)
            return code

        return wrapper

    nki_func = _make_nki_kernel()

    prim = jax.extend.core.Primitive("akima_exec")
    prim.multiple_results = True

    @prim.def_abstract_eval
    def _abs(*_, **__):
        return (jax.core.ShapedArray((ROWS, COLS), np.float32),)

    def _layouts(shapes):
        return [list(reversed(range(len(s)))) for s in shapes]

    def _lowering(ctx, *in_nodes):
        from neuronxcc.starfish.penguin.ir.NativeKernel import KERNEL_VERSION

        result_types = [mlir.aval_to_ir_type(a) for a in ctx.avals_out]
        code = raw_nki(nki_func)(list(ctx.avals_in))
        config = {
            "kernel_version": KERNEL_VERSION,
            "func_literal": code.serialize_ir_string("akima_kernel_ir"),
            "grid": [],
            "func_name": "akima_kernel",
            "has_collectives": False,
            "mac_count": 0,
            "tiled": False,
        }
        dumped = base64.b64encode(json.dumps(config).encode()).decode()
        return _mlir_custom_call(
            "AwsNeuronCustomNativeKernel",
            operands=list(in_nodes),
            result_types=result_types,
            operand_layouts=_layouts(a.shape for a in ctx.avals_in),
            result_layouts=_layouts(a.shape for a in ctx.avals_out),
            backend_config=dumped,
        ).results

    mlir.register_lowering(prim, _lowering, platform="neuron")

    devices = jax.devices()[:N_CORES]
    mesh = Mesh(np.asarray(devices), ("core",))

    def _body(x_shard, tab_shard):
        return prim.bind(x_shard, tab_shard)[0]

    sharded = jax.jit(shard_map(
        _body, mesh=mesh,
        in_specs=(PartitionSpec("core"), PartitionSpec("core")),
        out_specs=PartitionSpec("core"),
        check_rep=False,
    ))

    _EXEC_CACHE["exec"] = sharded
    return sharded


# ----------------------------------------------------------------------------
# Public entry point
# ----------------------------------------------------------------------------


def kernel(input: np.ndarray, value: np.ndarray) -> np.ndarray:
    input = np.ascontiguousarray(np.asarray(input, dtype=np.float32))
    value = np.asarray(value, dtype=np.float32)
    assert input.shape == (32, 1024, 1024), input.shape

    tab = _build_table(value)
    table = np.broadcast_to(tab, (ROWS, TAB_N)).copy()

    sharded = _build_executor()

    # shard on the leading dim: core i gets planes [4i, 4i+4)
    x_global = input.reshape(N_CORES * ROWS, COLS)
    tab_global = np.tile(table, (N_CORES, 1))

    out = sharded(x_global, tab_global)
    return np.asarray(out).reshape(32, 1024, 1024)


if __name__ == "__main__":
    inp = np.load("cache/input.npy")
    val = np.load("cache/value.npy")
    out = kernel(input=inp, value=val)
    exp = np.load("cache/expected.npy")
    err = out.astype(np.float64) - exp.astype(np.float64)
    print("rel_l2:", np.linalg.norm(err) / np.linalg.norm(exp))


# revision 23
# speedup vs baseline: 1.4128x; 1.1813x over previous
"""Akima spline interpolation kernel for Trainium2 (8 NeuronCores, data parallel).

Strategy:
  - The reference output is f(x) = akima_spline(x) for x in [0,1). Host
    fits an L2-optimal straight line to f on each of 511 uniform bins of
    width 1/510 centered at j/510 (bin j covers [(j-0.5)/510,
    (j+0.5)/510)): f(x) ~ A[j] + B[j]*w with w = 510*x - j in [-.5,.5).
    A and B are rounded to bf16 and packed into one uint32 per bin
    (A low half, B high half) - a 512-entry table, which fits the GPSIMD
    gather's 512-element pool-buffer window so each tile needs exactly
    one single-window gather. Measured end-to-end rel L2 error of this
    model (including bf16 rounding): ~5.1e-3 (budget 2e-2).
  - Device per tile: rbig = act(copy, x, scale=510, bias=M) rounds 510*x
    to the nearest integer via the magic constant M = 1.5*2^23; two more
    act copies with bias=-M produce the index as uint32 and as float;
    one gather fetches the packed word; the vector engine computes
    w = 510*x - idxf (scalar_tensor_tensor) and evaluates A + B*w as two
    tensor_tensor ops on stride-2 bf16 views of the gathered word.
    Per tile: scalar 3 ops, vector 3 ops, GPSIMD 1 gather - roughly
    balanced, with the gather the slight bottleneck.
  - Sharding: pure data parallel on the leading dim (4 of 32 planes per
    core); the 2KB table is replicated to all partitions of every core.
"""
import base64
import json
import sys

import numpy as np

if "/opt/trn_rl_repo" not in sys.path:
    sys.path.insert(0, "/opt/trn_rl_repo")

NODES = 256
N_CORES = 8
ROWS = 128
COLS = 4 * 1024 * 1024 // ROWS  # per-core shard [128, 32768]
F_TILE = 2048
NSEG = 510                      # bins per unit; table has NSEG+1 entries
TAB_N = 512                     # padded table length (= pool buffer window)
MAGIC = float(np.float32(1.5 * 2.0 ** 23))

# ----------------------------------------------------------------------------
# Host-side table construction
# ----------------------------------------------------------------------------


def _akima_slopes_f64(value):
    h = 1.0 / (NODES - 1)
    v = value.astype(np.float64)
    m = (v[1:] - v[:-1]) / h
    m_m1 = 2.0 * m[0] - m[1]
    m_m2 = 2.0 * m_m1 - m[0]
    m_p1 = 2.0 * m[-1] - m[-2]
    m_p2 = 2.0 * m_p1 - m[-1]
    me = np.concatenate([[m_m2, m_m1], m, [m_p1, m_p2]])
    w1 = np.abs(me[3:] - me[2:-1])
    w2 = np.abs(me[1:-2] - me[:-3])
    mi_1 = me[1:-2]
    mi = me[2:-1]
    denom = w1 + w2
    safe = np.where(denom > 0, denom, 1.0)
    return np.where(denom > 0, (w1 * mi_1 + w2 * mi) / safe, 0.5 * (mi_1 + mi))


def _spline_f64(x, value, s):
    h = 1.0 / (NODES - 1)
    v = value.astype(np.float64)
    x = np.clip(x, 0.0, 1.0)
    t = x / h
    idx = np.clip(np.floor(t).astype(np.int64), 0, NODES - 2)
    u = t - idx
    v0 = v[idx]
    v1 = v[idx + 1]
    s0 = s[idx]
    s1 = s[idx + 1]
    u2 = u * u
    u3 = u2 * u
    return ((2 * u3 - 3 * u2 + 1) * v0 + (u3 - 2 * u2 + u) * h * s0
            + (-2 * u3 + 3 * u2) * v1 + (u3 - u2) * h * s1)


def _build_table(value):
    import ml_dtypes
    s = _akima_slopes_f64(value)
    SS = 64
    j = np.arange(NSEG + 1)
    offs = (np.arange(SS) + 0.5) / SS - 0.5
    xs = (j[:, None] + offs[None, :]) / NSEG
    xs = np.clip(xs, 0.0, 1.0 - 1e-12)
    f = _spline_f64(xs.ravel(), value, s).reshape(NSEG + 1, SS)
    A = f.mean(axis=1)
    B = 12.0 * (f * offs[None, :]).mean(axis=1)
    Ab = A.astype(ml_dtypes.bfloat16).view(np.uint16).astype(np.uint32)
    Bb = B.astype(ml_dtypes.bfloat16).view(np.uint16).astype(np.uint32)
    tab = np.zeros(TAB_N, dtype=np.uint32)
    tab[:NSEG + 1] = Ab | (Bb << 16)
    return tab


# ----------------------------------------------------------------------------
# NKI kernel
# ----------------------------------------------------------------------------


def _make_nki_kernel():
    import neuronxcc.nki.language as nl
    import neuronxcc.nki.isa as nisa
    from neuronxcc.nki.isa.constants import dge_mode

    n_tiles = COLS // F_TILE

    def akima_kernel(inputs):
        x, table = inputs[0], inputs[1]
        out = nl.ndarray(shape=[ROWS, COLS], dtype=nl.float32, buffer=nl.shared_hbm)
        tab_sb = nl.load(table)
        i_p = nl.arange(ROWS)[:, None]
        i_f = nl.arange(F_TILE)[None, :]
        magic_bias = nisa.memset((ROWS, 1), MAGIC, nl.float32)
        neg_magic_bias = nisa.memset((ROWS, 1), -MAGIC, nl.float32)

        # Explicit rotating SBUF buffers: without them the allocator's
        # address reuse creates WAR hazards that serialize consecutive
        # tiles.
        NBUF = 3

        def mkbufs():
            return dict(
                x=nl.ndarray(shape=[ROWS, F_TILE], dtype=nl.float32, buffer=nl.sbuf),
                rbig=nl.ndarray(shape=[ROWS, F_TILE], dtype=nl.float32, buffer=nl.sbuf),
                idx=nl.ndarray(shape=[ROWS, F_TILE], dtype=nl.uint32, buffer=nl.sbuf),
                g=nl.ndarray(shape=[ROWS, F_TILE], dtype=nl.uint32, buffer=nl.sbuf),
                r=nl.ndarray(shape=[ROWS, F_TILE], dtype=nl.float32, buffer=nl.sbuf),
            )

        bufs = [mkbufs() for _ in range(NBUF)]
        # idxf, w and m live in PSUM: GPSIMD has no PSUM port, so
        # vector-engine ops with a PSUM operand need only one SBUF port
        # and stop colliding with the gather for the shared POOL/DVE SBUF
        # port. The scalar engine writes idxf full-width (ACT supports 4K
        # free dim to PSUM); the vector stage runs in 512-f32 one-bank
        # chunks. Single-buffered w/m are safe: the vector engine executes
        # its ops in order, so each chunk's writer issues after the
        # previous chunk's readers.
        PCH = 512
        idxf_ps = [nl.ndarray(shape=[ROWS, PCH], dtype=nl.float32, buffer=nl.psum)
                   for _ in range(2)]
        w_ps = nl.ndarray(shape=[ROWS, PCH], dtype=nl.float32, buffer=nl.psum)
        m_ps = nl.ndarray(shape=[ROWS, PCH], dtype=nl.float32, buffer=nl.psum)
        i_c = nl.arange(PCH)[None, :]

        for t in range(n_tiles):
            B = bufs[t % NBUF]
            sl = slice(t * F_TILE, (t + 1) * F_TILE)
            # HWDGE DMAs: keep descriptor generation off GPSIMD (SWDGE's
            # Q7 descgen and descriptor rings contend with the gather and
            # with 2-port vector ops for SBUF ports).
            nisa.dma_copy(src=x[:, sl], dst=B['x'][i_p, i_f],
                          dge_mode=dge_mode.hwdge)
            x_sb = B['x'][i_p, i_f]
            B['rbig'][i_p, i_f] = nisa.activation(
                np.copy, x_sb, bias=magic_bias, scale=float(NSEG))
            B['idx'][i_p, i_f] = nisa.activation(
                np.copy, B['rbig'][i_p, i_f], bias=neg_magic_bias, dtype=nl.uint32)
            B['g'][i_p, i_f] = nl.gather_flattened(
                data=tab_sb, indices=B['idx'][i_p, i_f])
            gb = B['g'].view(nl.bfloat16)  # [P, 2F]: A at even, B at odd
            for c in range(F_TILE // PCH):
                cf = c * PCH + i_c
                ixp = idxf_ps[c % 2]
                ixp[i_p, i_c] = nisa.activation(
                    np.copy, B['rbig'][i_p, cf], bias=neg_magic_bias)
                w_ps[i_p, i_c] = nisa.scalar_tensor_tensor(
                    data=B['x'][i_p, cf], op0=np.multiply,
                    operand0=float(NSEG), op1=np.subtract,
                    operand1=ixp[i_p, i_c])
                m_ps[i_p, i_c] = nisa.tensor_tensor(
                    gb[i_p, cf * 2 + 1], w_ps[i_p, i_c], np.multiply,
                    dtype=nl.float32)
                B['r'][i_p, cf] = nisa.tensor_tensor(
                    m_ps[i_p, i_c], gb[i_p, cf * 2], np.add,
                    dtype=nl.float32)
            nisa.dma_copy(src=B['r'][i_p, i_f], dst=out[:, sl],
                          dge_mode=dge_mode.hwdge)
        return [out]

    return akima_kernel


# ----------------------------------------------------------------------------
# jax integration (AwsNeuronCustomNativeKernel custom call, SPMD over 8 cores)
# ----------------------------------------------------------------------------

_EXEC_CACHE = {}


def _build_executor():
    if "exec" in _EXEC_CACHE:
        return _EXEC_CACHE["exec"]

    import functools
    import jax
    from jax.interpreters import mlir
    from jax._src.interpreters.mlir import custom_call as _mlir_custom_call
    from jax.sharding import Mesh, PartitionSpec
    from jax.experimental.shard_map import shard_map
    from concourse.bass2jax import install_neuronx_cc_hook

    install_neuronx_cc_hook()

    def raw_nki(func):
        # concourse.nki.raw_nki with platform_target='trn2' (the default
        # CompileOpts says trn1, which rejects HWDGE dma_copy).
        from neuronxcc.nki.compiler.backends.neuron.CompileOpts import CompileOpts
        from neuronxcc.nki.compiler.backends.neuron.KernelBuilder import NeuronCodegen
        from neuronxcc.nki.compiler.backends.neuron.nki_ctx import nki_ctx
        from neuronxcc.nki.compiler.backends.neuron.tensors import TensorRef
        from neuronxcc.starfish.penguin.ir.Function import Function
        from neuronxcc.starfish.penguin.ir.OptLevel import OptLevel

        @functools.wraps(func)
        def wrapper(inputs):
            code = Function(name="func", opt_level=OptLevel.default_level)
            bb = code.addBasicBlock()
            opts = CompileOpts(platform_target="trn2")
            with NeuronCodegen.new_ctx(cu=code, curstmt=bb, opts=opts) as ctx:
                with ctx.kernel_scope(
                    ctx.function, py_func=func, spmd_block=ctx.builder.curstmt
                ) as scope:
                    nki_inputs = []
                    for i, inp in enumerate(inputs):
                        tensor = nki_ctx().add_parameter(
                            name=f"input{i}",
                            shape=list(inp.shape),
                            dtype=inp.dtype,
                            is_mutable=False,
                        )
                        tensor.isInput = True
                        nki_inputs.append(TensorRef(tensor))
                    outputs = func(nki_inputs)
                    scope.add_kernel_return_values(list(outputs))
                ctx.finalize_kernel(scope)
            return code

        return wrapper

    nki_func = _make_nki_kernel()

    prim = jax.extend.core.Primitive("akima_exec")
    prim.multiple_results = True

    @prim.def_abstract_eval
    def _abs(*_, **__):
        return (jax.core.ShapedArray((ROWS, COLS), np.float32),)

    def _layouts(shapes):
        return [list(reversed(range(len(s)))) for s in shapes]

    def _lowering(ctx, *in_nodes):
        from neuronxcc.starfish.penguin.ir.NativeKernel import KERNEL_VERSION

        result_types = [mlir.aval_to_ir_type(a) for a in ctx.avals_out]
        code = raw_nki(nki_func)(list(ctx.avals_in))
        config = {
            "kernel_version": KERNEL_VERSION,
            "func_literal": code.serialize_ir_string("akima_kernel_ir"),
            "grid": [],
            "func_name": "akima_kernel",
            "has_collectives": False,
            "mac_count": 0,
            "tiled": False,
        }
        dumped = base64.b64encode(json.dumps(config).encode()).decode()
        return _mlir_custom_call(
            "AwsNeuronCustomNativeKernel",
            operands=list(in_nodes),
            result_types=result_types,
            operand_layouts=_layouts(a.shape for a in ctx.avals_in),
            result_layouts=_layouts(a.shape for a in ctx.avals_out),
            backend_config=dumped,
        ).results

    mlir.register_lowering(prim, _lowering, platform="neuron")

    devices = jax.devices()[:N_CORES]
    mesh = Mesh(np.asarray(devices), ("core",))

    def _body(x_shard, tab_shard):
        return prim.bind(x_shard, tab_shard)[0]

    sharded = jax.jit(shard_map(
        _body, mesh=mesh,
        in_specs=(PartitionSpec("core"), PartitionSpec("core")),
        out_specs=PartitionSpec("core"),
        check_rep=False,
    ))

    _EXEC_CACHE["exec"] = sharded
    return sharded


# ----------------------------------------------------------------------------
# Public entry point
# ----------------------------------------------------------------------------


def kernel(input: np.ndarray, value: np.ndarray) -> np.ndarray:
    input = np.ascontiguousarray(np.asarray(input, dtype=np.float32))
    value = np.asarray(value, dtype=np.float32)
    assert input.shape == (32, 1024, 1024), input.shape

    tab = _build_table(value)
    table = np.broadcast_to(tab, (ROWS, TAB_N)).copy()

    sharded = _build_executor()

    # shard on the leading dim: core i gets planes [4i, 4i+4)
    x_global = input.reshape(N_CORES * ROWS, COLS)
    tab_global = np.tile(table, (N_CORES, 1))

    out = sharded(x_global, tab_global)
    return np.asarray(out).reshape(32, 1024, 1024)


if __name__ == "__main__":
    inp = np.load("cache/input.npy")
    val = np.load("cache/value.npy")
    out = kernel(input=inp, value=val)
    exp = np.load("cache/expected.npy")
    err = out.astype(np.float64) - exp.astype(np.float64)
    print("rel_l2:", np.linalg.norm(err) / np.linalg.norm(exp))
